# revision 13
# baseline (speedup 1.0000x reference)
"""Multi-head attention layer (L=2048, B=2, D=1024, H=16) on 8 Trainium2 cores.

Sharding: batch*heads across cores — core c handles batch c//4, heads
4*(c%4)..4*(c%4)+4.  Tensor-parallel W_in column slice (per-head) and W_out
row slice; per-core partial outputs are summed on the host (2 groups of 4).

v15: PE-roofline schedule with SINGLE-HEAD attention blocks.  Per key-chunk
(mc) iteration the psum tag "S" sees only [fill, S] tiles, so the 2-slot
rotation never waits on a hot exp; z is one [P,1024] tile per block (bufs=2
for cross-block overlap).  All projection / output-projection work is
injected as fine-grained fills (128-col qk pieces, v chunks, 512-col
out-proj PARTIALS split by W_out row-pair so late blocks have fill too).
AV(mc) is issued at iteration mc+2 so it never waits on its exp.  Fill
copies, out-proj combines and norm multiplies ride the idle Pool (gpsimd)
engine.  All matmul operands are f16 (same 11-bit mantissa as f32r, full PE
rate at every width).  Striped DMA lets the first matmul start at ~3us; the
last sub-block is 256 queries to shrink the exp->AV->norm->out tail.
"""

import sys

for _p in ("/opt/trn_rl_repo",):
    if _p not in sys.path:
        sys.path.append(_p)

import numpy as np

L, B, D, H = 2048, 2, 1024, 16
HD = 64
NCORES = 8
HPC = 4              # heads per core
J = HPC * HD         # 256 per-core head-dim slice
KC = D // 128        # 8 contraction chunks
P = 128

_COMPILED = None


def _build():
    import concourse.bacc as bacc
    import concourse.mybir as mybir
    import concourse.tile as tile
    from contextlib import ExitStack

    f32 = mybir.dt.float32
    f16 = mybir.dt.float16
    Exp = mybir.ActivationFunctionType.Exp
    Mult = mybir.AluOpType.mult
    Add = mybir.AluOpType.add

    nc = bacc.Bacc("TRN2", target_bir_lowering=False, debug=False)

    xT_d = nc.dram_tensor("xT", (D, L), f16, kind="ExternalInput")
    wqk_d = nc.dram_tensor("wqkT", (D, 2 * J), f16, kind="ExternalInput")
    wv_d = nc.dram_tensor("wvT", (D, J), f16, kind="ExternalInput")
    wo_d = nc.dram_tensor("woT", (J, D), f16, kind="ExternalInput")
    ident_d = nc.dram_tensor("ident", (P, P), f16, kind="ExternalInput")
    out_d = nc.dram_tensor("out_p", (L, D), f16, kind="ExternalOutput")

    with tile.TileContext(nc) as tc, ExitStack() as ctx:
        pers = ctx.enter_context(tc.tile_pool(name="pers", bufs=1))
        psum = ctx.enter_context(tc.tile_pool(name="psum", bufs=2, space="PSUM"))
        att = ctx.enter_context(tc.tile_pool(name="att", bufs=3))

        qk_sb = pers.tile([P, 4, L], f16)           # jc 0,1: q pairs; 2,3: k pairs
        v_sb = pers.tile([P, 16, HPC, P], f16)      # ones cols 0:64, v 64:128
        zn_sb = pers.tile([P, 2, L], f16)           # normalized z^T per pair
        wo_sb = pers.tile([P, 2, D], f16)
        xT_sb = pers.tile([P, KC, L], f16)
        wqk_sb = pers.tile([P, KC, 2 * J], f16)
        wv_sb = pers.tile([P, KC, J], f16)
        id_sb = pers.tile([P, P], f16)

        out_ap = out_d.ap().rearrange("(t p) o -> p t o", p=P)
        xT_ap = xT_d.ap().rearrange("(kc p) m -> p kc m", p=P)
        wqk_ap = wqk_d.ap().rearrange("(kc p) j -> p kc j", p=P)
        wv_ap = wv_d.ap().rearrange("(kc p) j -> p kc j", p=P)
        wo_ap = wo_d.ap().rearrange("(dc p) o -> p dc o", p=P)

        # stripe DMAs ordered for earliest prologue start
        nc.sync.dma_start(wqk_sb[:, :, 0 * P:1 * P], wqk_ap[:, :, 0 * P:1 * P])
        nc.scalar.dma_start(xT_sb[:, 0:4, 0:512], xT_ap[:, 0:4, 0:512])
        nc.scalar.dma_start(xT_sb[:, 4:8, 0:512], xT_ap[:, 4:8, 0:512])
        nc.sync.dma_start(wqk_sb[:, :, 2 * P:3 * P], wqk_ap[:, :, 2 * P:3 * P])
        nc.sync.dma_start(xT_sb[:, 0:4, 512:1024], xT_ap[:, 0:4, 512:1024])
        nc.scalar.dma_start(xT_sb[:, 4:8, 512:1024], xT_ap[:, 4:8, 512:1024])
        nc.sync.dma_start(wv_sb[:], wv_ap[:])
        nc.scalar.dma_start(wqk_sb[:, :, 1 * P:2 * P], wqk_ap[:, :, 1 * P:2 * P])
        nc.sync.dma_start(wqk_sb[:, :, 3 * P:4 * P], wqk_ap[:, :, 3 * P:4 * P])
        nc.scalar.dma_start(xT_sb[:, 0:4, 1024:1536], xT_ap[:, 0:4, 1024:1536])
        nc.sync.dma_start(xT_sb[:, 4:8, 1024:1536], xT_ap[:, 4:8, 1024:1536])
        nc.scalar.dma_start(xT_sb[:, 0:4, 1536:2048], xT_ap[:, 0:4, 1536:2048])
        nc.sync.dma_start(xT_sb[:, 4:8, 1536:2048], xT_ap[:, 4:8, 1536:2048])
        nc.scalar.dma_start(wo_sb[:], wo_ap[:])
        nc.sync.dma_start(id_sb[:], ident_d.ap())

        # ---- fill work units (copies on Pool/gpsimd) ----
        def qk_chunk(jc, c0, w=128):
            pt = psum.tile([P, w], f32, tag="S", bufs=3, name=f"qkp_{jc}_{c0}")
            for kc in range(KC):
                nc.tensor.matmul(
                    pt[:],
                    wqk_sb[:, kc, jc * P:(jc + 1) * P],
                    xT_sb[:, kc, c0:c0 + w],
                    start=(kc == 0), stop=(kc == KC - 1),
                )
            nc.vector.tensor_copy(qk_sb[:, jc, c0:c0 + w], pt[:])

        def v_chunk(mc):
            pt = psum.tile([P, J], f32, tag="S", bufs=3, name=f"vp_{mc}")
            for kc in range(KC):
                nc.tensor.matmul(
                    pt[:],
                    xT_sb[:, kc, mc * P:(mc + 1) * P],
                    wv_sb[:, kc, :],
                    start=(kc == 0), stop=(kc == KC - 1),
                )
            nc.vector.tensor_copy(
                v_sb[:, mc, :, 0:64],
                pt[:].rearrange("p (h e) -> p h e", e=64),
            )

        def out_half(t, oc):
            po = psum.tile([P, 512], f32, tag="S", bufs=3, name=f"po_{t}_{oc}")
            for dc in range(2):
                nc.tensor.matmul(
                    po[:],
                    zn_sb[:, dc, t * P:(t + 1) * P],
                    wo_sb[:, dc, oc * 512:(oc + 1) * 512],
                    start=(dc == 0), stop=(dc == 1),
                )
            tag = "o" if oc == 0 else "o2"
            ot = att.tile([P, 512], f16, tag=tag, bufs=4, name=f"ot_{t}_{oc}")
            nc.vector.tensor_copy(ot[:], po[:])
            nc.sync.dma_start(out_ap[:, t, oc * 512:(oc + 1) * 512], ot[:])

        po0_sb = {}

        def out_dc0(t, oc):
            po = psum.tile([P, 512], f32, tag="S", bufs=3, name=f"pod0_{t}_{oc}")
            nc.tensor.matmul(
                po[:],
                zn_sb[:, 0, t * P:(t + 1) * P],
                wo_sb[:, 0, oc * 512:(oc + 1) * 512],
                start=True, stop=True,
            )
            sb = att.tile([P, 512], f16, tag="po0", bufs=16, name=f"po0_{t}_{oc}")
            nc.vector.tensor_copy(sb[:], po[:])
            po0_sb[(t, oc)] = sb

        def out_rest(t, oc, cp="act", dge=None):
            po = psum.tile([P, 512], f32, tag="S", bufs=3, name=f"pod1_{t}_{oc}")
            p0 = po0_sb.pop((t, oc))
            if cp != "dveadd":
                nc.tensor.matmul(po[:], id_sb[:], p0[:], start=True, stop=False)
            nc.tensor.matmul(
                po[:],
                zn_sb[:, 1, t * P:(t + 1) * P],
                wo_sb[:, 1, oc * 512:(oc + 1) * 512],
                start=(cp == "dveadd"), stop=True,
            )
            tag = "o" if oc == 0 else "o2"
            ot = att.tile([P, 512], f16, tag=tag, bufs=4, name=f"ot_{t}_{oc}")
            if cp == "act":
                nc.scalar.copy(ot[:], po[:])
            elif cp == "dveadd":
                nc.vector.tensor_tensor(ot[:], po[:], p0[:], Add)
            else:
                nc.vector.tensor_copy(ot[:], po[:])
            (dge or nc.sync).dma_start(out_ap[:, t, oc * 512:(oc + 1) * 512], ot[:])

        # ones columns 0:64 for every head — keeps softmax row-sums on psum
        # partitions 0-63 where the custom-DVE reciprocal is valid.
        ones_sc = pers.tile([P, 64], f32)
        nc.vector.memset(ones_sc[:], 1.0)
        for h in range(HPC):
            nc.vector.tensor_copy(
                v_sb[:, :, h, 64:65],
                ones_sc[:, None, 0:1].to_broadcast((P, 16, 1)),
            )

        # ---- prologue: q0/k0 first 512 tokens, kc-interleaved pair ----
        pts = {}
        for jc in (0, 2):
            pts[jc] = psum.tile([P, 512], f32, tag="S", bufs=3, name=f"qkp_{jc}_pro")
        for half in range(2):
            for jc in (0, 2):
                for kc in range(4 * half, 4 * half + 4):
                    nc.tensor.matmul(
                        pts[jc][:],
                        wqk_sb[:, kc, jc * P:(jc + 1) * P],
                        xT_sb[:, kc, 0:512],
                        start=(kc == 0), stop=(kc == KC - 1),
                    )
        for jc in (0, 2):
            nc.vector.tensor_copy(qk_sb[:, jc, 0:512], pts[jc][:])

        # ---- single-head attention block with fills + deferred AV ----
        pending_tr = []

        def attn_block(h, l0, qn, fills):
            for i, fn in enumerate(pending_tr):
                fills.setdefault(1 + 2 * i, []).append(fn)
            pending_tr.clear()
            hp = h // 2
            r0 = (h % 2) * 64
            nq2 = (qn + 511) // 512
            nqc = qn // P
            zt = psum.tile([P, nqc, P], f32, tag="z", bufs=1, name=f"z_{h}_{l0}")
            Es = {}

            def do_av(mc):
                # one accumulation group per PSUM bank (4 qc-chunks/bank):
                # start zeroes the bank on its first write; stop on the last
                E = Es.pop(mc)
                for qc in range(nqc):
                    nc.tensor.matmul(
                        zt[:, qc, 0:65],
                        E[:, qc * P:(qc + 1) * P],
                        v_sb[:, mc, h, 0:65],
                        start=(mc == 0 and qc % 4 == 0),
                        stop=(mc == 15 and (qc % 4 == 3 or qc == nqc - 1)),
                    )

            for mc in range(16):
                S = psum.tile([P, qn], f32, tag="S", bufs=3, name=f"S_{h}_{l0}_{mc}")
                for q2 in range(nq2):
                    w = min(512, qn - q2 * 512)
                    nc.tensor.matmul(
                        S[:, q2 * 512:q2 * 512 + w],
                        qk_sb[r0:r0 + 64, 2 + hp, mc * P:(mc + 1) * P],
                        qk_sb[r0:r0 + 64, hp, l0 + q2 * 512:l0 + q2 * 512 + w],
                        start=True, stop=True,
                    )
                E = att.tile([P, qn], f16, tag="E", bufs=8, name=f"E_{h}_{l0}_{mc}")
                nc.scalar.activation(E[:], S[:], Exp, scale=0.125)
                Es[mc] = E
                for fn in fills.get(mc, ()):
                    fn()
                if mc >= 5:
                    do_av(mc - 5)
            for m in (11, 12, 13, 14, 15):
                do_av(m)

            # normalize token-major: r = 1/sums (col 64), zn_tm = z * r
            rsb = att.tile([P, 16], f32, tag="r", bufs=2, name=f"r_{h}_{l0}")
            nc.vector.reciprocal_approx_fast(out=rsb[:, 0:nqc], in_=zt[:, :, 64])
            zn_tm = att.tile([P, 16, 64], f16, tag="ztm", bufs=2, name=f"ztm_{h}_{l0}")
            nc.vector.tensor_tensor(
                zn_tm[:, 0:nqc, :], zt[:, :, 0:64],
                rsb[:, 0:nqc, None].to_broadcast((P, nqc, 64)), Mult,
            )

            # PE transpose back to dim-major (zn^T = znT @ I) — deferred into
            # the next block so the PE never waits on the DVE mult here
            def transpose_group(g):
                n = min(4, nqc - g * 4)
                tr = psum.tile([P, 512], f32, tag="S", bufs=3, name=f"tr_{h}_{l0}_{g}")
                for j in range(n):
                    nc.tensor.matmul(
                        tr[0:64, j * P:(j + 1) * P],
                        zn_tm[:, g * 4 + j, :],
                        id_sb[:],
                        start=True, stop=True,
                    )
                nc.vector.tensor_copy(
                    zn_sb[r0:r0 + 64, hp, l0 + g * 512:l0 + g * 512 + n * P],
                    tr[0:64, 0:n * P],
                )

            for g in range((nqc + 3) // 4):
                pending_tr.append(lambda g=g: transpose_group(g))

        def spread(pieces, stride=2):
            """Front-load two pieces at mc0/mc1 (rides out the previous
            block's ACT backlog), then spread the rest."""
            d = {}
            for i, fn in enumerate(pieces):
                if i < 2:
                    mc = i
                elif stride > 1:
                    mc = 2 + (i - 2) * stride
                else:
                    mc = i
                d.setdefault(min(mc, 15), []).append(fn)
            return d

        # A1 (h0, q 0:512): v chunks + q0 tail + k0 tail (k0 piece for S(mc)
        # lands at mc-2)
        kq = (
            [(0, 512), (0, 640)]
            + [(2, 512 + 128 * i) for i in range(12)]     # k0 512:2048
            + [(0, 768), (0, 896)]
        )
        a1 = {0: [lambda: qk_chunk(*kq[0])], 1: [lambda: qk_chunk(*kq[1])]}
        for mc in range(2, 16):
            jc, c0 = kq[mc]
            a1[mc] = [
                lambda m=mc - 2: v_chunk(m),
                lambda jc=jc, c0=c0: qk_chunk(jc, c0),
            ]
        a1[15].append(lambda: v_chunk(14))
        a1[15].append(lambda: v_chunk(15))
        attn_block(0, 0, 512, a1)

        # A2 (h1, 0:1024): k1 half0
        attn_block(1, 0, 1024,
                   spread([lambda c0=128 * i: qk_chunk(3, c0) for i in range(8)]
                          + [lambda c0=1024 + 128 * i: qk_chunk(0, c0) for i in range(8)],
                          stride=1))
        # A3 (h0, 512:1024): q1 half0 (A4 reads it from mc0)
        attn_block(0, 512, 512,
                   spread([lambda c0=128 * i: qk_chunk(1, c0) for i in range(8)]))
        # A4 (h2, 0:1024): k1 half1 (self-feed: piece j lands well before
        # its S(mc 8+j) use)
        attn_block(2, 0, 1024,
                   spread([lambda c0=1024 + 128 * i: qk_chunk(3, c0) for i in range(8)]
                          + [lambda c0=1024 + 128 * i: qk_chunk(1, c0) for i in range(8)],
                          stride=1))
        # A5 (h3, 0:1024): q0 half1 (for A6) + q1 half1 (for A7)
        attn_block(3, 0, 1024, {})
        # A6 (h0, 1024:2048): out t0..3 (all heads of half0 done after A5);
        # start at mc1 so A5's norm has landed
        a6 = {}
        for i in range(8):
            t, oc = i // 2, i % 2
            a6.setdefault(6 + (9 * i) // 8, []).append(
                lambda t=t, oc=oc: out_half(t, oc))
        attn_block(0, 1024, 1024, a6)
        # A7 (h1, 1024:2048): out t4..7
        a7 = {}
        for i in range(4):
            t, oc = 4 + i // 2, i % 2
            a7[4 + 3 * i] = [lambda t=t, oc=oc: out_half(t, oc)]
        attn_block(1, 1024, 1024, a7)
        # A8 (h2, 1024:2048): out dc0 partials t8..15 (pair0 half1 ready)
        a8 = {}
        for i in range(4):
            t, oc = 6 + i // 2, i % 2
            a8[3 * i] = [lambda t=t, oc=oc: out_half(t, oc)]
        for i in range(8):
            t, oc = 8 + i // 2, i % 2
            a8.setdefault(8 + (7 * i) // 8, []).append(
                lambda t=t, oc=oc: out_dc0(t, oc))
        attn_block(2, 1024, 1024, a8)
        # A9 (h3, 1024:1792): dc0 partials t12..15 (pair0 half1 done at A7)
        a9 = {}
        for i in range(8):
            t, oc = 12 + i // 2, i % 2
            a9[1 + 2 * i if i < 7 else 15] = [lambda t=t, oc=oc: out_dc0(t, oc)]
        attn_block(3, 1024, 768, a9)
        # A10 (h3, 1792:2048): out rest t8..13 (h2 full, h3 to 1792 ready);
        # start at mc1 so A9's norm has landed
        a10 = {}
        for i in range(12):
            t, oc = 8 + i // 2, i % 2
            cp = "act" if i % 3 == 2 else "dveadd"
            a10.setdefault(8 + (7 * i) // 12, []).append(
                lambda t=t, oc=oc, cp=cp: out_rest(t, oc, cp))
        attn_block(3, 1792, 256, a10)

        # tail: flush last block's transposes, then out t14, t15
        for fn in pending_tr:
            fn()
        pending_tr.clear()
        for t in (14, 15):
            out_rest(t, 0, "act", nc.scalar)
            out_rest(t, 1, "dveadd", nc.sync)

    nc.compile()
    return nc


def _get_compiled():
    global _COMPILED
    if _COMPILED is None:
        _COMPILED = _build()
    return _COMPILED


def _shard_inputs(x, W_in, W_out):
    in_maps = []
    xTs = [x[:, b, :].T.astype(np.float16) for b in range(B)]
    for c in range(NCORES):
        b = c // 4
        lo = (c % 4) * J
        Wq = W_in[lo:lo + J]
        Wk = W_in[D + lo:D + lo + J]
        Wv = W_in[2 * D + lo:2 * D + lo + J]
        in_maps.append({
            "xT": xTs[b],
            "wqkT": np.concatenate([Wq, Wk], 0).T.astype(np.float16),
            "wvT": Wv.T.astype(np.float16),
            "woT": np.ascontiguousarray(W_out[:, lo:lo + J].T).astype(np.float16),
            "ident": np.eye(P, dtype=np.float16),
        })
    return in_maps


def _reference_numpy(q, mask, W_in, b_in, W_out, b_out, num_heads):
    l, b, d = q.shape
    hd = d // num_heads
    qkv = q.reshape(l * b, d) @ W_in.T + b_in
    qkv = qkv.reshape(l, b, 3 * d)
    qh, kh, vh = np.split(qkv, 3, axis=-1)

    def to_heads(t):
        return t.reshape(l, b * num_heads, hd).transpose(1, 0, 2)

    qh, kh, vh = to_heads(qh), to_heads(kh), to_heads(vh)
    qh = qh / np.sqrt(np.float32(hd))
    scores = np.einsum("nld,nmd->nlm", qh, kh) + mask
    scores -= scores.max(axis=-1, keepdims=True)
    e = np.exp(scores)
    attn = e / e.sum(axis=-1, keepdims=True)
    z = np.einsum("nlm,nmd->nld", attn, vh)
    z = z.transpose(1, 0, 2).reshape(l * b, d)
    z = z @ W_out.T + b_out
    return z.reshape(l, b, d).astype(np.float32)


def kernel(q, k, v, mask, W_in, b_in, W_out, b_out, num_heads):
    num_heads = int(num_heads)
    q = np.asarray(q, dtype=np.float32)
    W_in = np.asarray(W_in, dtype=np.float32)
    W_out = np.asarray(W_out, dtype=np.float32)
    b_in = np.asarray(b_in, dtype=np.float32)
    b_out = np.asarray(b_out, dtype=np.float32)
    mask = np.asarray(mask, dtype=np.float32)

    if (
        num_heads != H
        or q.shape != (L, B, D)
        or W_in.shape != (3 * D, D)
        or W_out.shape != (D, D)
        or np.any(mask)
        or np.any(b_in)
    ):
        return _reference_numpy(q, mask, W_in, b_in, W_out, b_out, num_heads)

    from concourse import bass_utils

    nc = _get_compiled()
    in_maps = _shard_inputs(q, W_in, W_out)
    res = bass_utils.run_bass_kernel_spmd(
        nc, in_maps, core_ids=list(range(NCORES))
    )

    out = np.zeros((L, B, D), dtype=np.float32)
    for c in range(NCORES):
        out[:, c // 4, :] += res.results[c]["out_p"].astype(np.float32)
    out += b_out
    return out
